# revision 11
# baseline (speedup 1.0000x reference)
"""Trainium2 Bass kernel for Mobile2Former cross-attention block.

Computation (per batch b):
    xf   = x[b].reshape(C, H*W)                      # [64, 3136] keys=values
    q    = (z[b] @ Wq + bq).reshape(heads, M, C)     # [8, 6, 64]
    attn = softmax(q @ xf * C**-0.5, axis=-1)        # [8, 6, 3136]
    res  = attn @ xf.T                               # [8, 6, 64]
    out  = res.transpose(1,0,2).reshape(M, -1) @ Wo + bo + z[b]

Strategy: data-parallel over B across 8 cores (16 batches/core), batches
processed in pairs (two batches stacked on the 128 SBUF partitions, C=64
each).  QK^T is computed directly in transposed layout (attn^T[n, hm]) by
using xf chunks (fp8) as the matmul stationary operand.  The AV matmul
consumes a host-pretransposed copy of x (fp8, with a ones column baked in
per chunk for the softmax denominator) as the moving operand, so no PE
transposes are needed.  AV and the output projection run in fp8 DoubleRow
mode (two 128-chunks contracted per matmul).  Softmax runs without max
subtraction (logits are O(1)); exp() applies the attention scale (folded
with an 8x fp8-range rescale of Wq) and writes fp8 attention weights
straight into a per-pair contiguous stationary buffer.
"""

import sys
from contextlib import ExitStack

import numpy as np

sys.path.insert(0, "/opt/trn_rl_repo")

import concourse.bass as bass
import concourse.tile as tile
from concourse import bacc as bacc_mod
from concourse import mybir
from concourse.bass_utils import run_bass_kernel_spmd

import ml_dtypes

BF16 = ml_dtypes.bfloat16
FP8 = ml_dtypes.float8_e4m3

N_CORES = 8
B, C, H, W = 128, 64, 56, 56
HW = H * W  # 3136
M, D = 6, 192
NH = 8
INNER = NH * C  # 512
BPC = B // N_CORES  # 16 batches per core
NPAIR = BPC // 2  # 8 pairs per core
NCHUNK = (HW + 127) // 128  # 25 (24 full + one 64-wide)
CSTRIDE = 144  # xt per-chunk col stride: 128 data + 1 ones + pad (16-mult)
XTW = NCHUNK * CSTRIDE  # 3600
XXW = HW + XTW  # 6736: packed [xf 3136 | xt 3600] per pair
W0 = 640  # first-wave xf piece loaded separately for fast start
QSC = 8.0  # Wq rescale so fp8 weights stay in normal range
ESC = float(C ** -0.5) / QSC  # exp() pre-scale: attn scale / QSC
# Schraudolph exp: exp(x*ESC) ~= bitcast_f32(int32(SCH_A*x + SCH_B))
SCH_A = (2.0 ** 23) / 0.6931471805599453 * ESC
SCH_B = 127.0 * 2 ** 23 - 366393.0  # min-RMS variant (~1.8% rel RMS)

F32 = mybir.dt.float32
I32 = mybir.dt.int32
BF = mybir.dt.bfloat16
F8 = mybir.dt.float8e4
DR = mybir.MatmulPerfMode.DoubleRow

_CACHE = {}

# waves of QK/exp chunks; last wave has the 64-wide chunk
WAVES = [(0, 5), (5, 5), (10, 5), (15, 5), (20, 5)]
NW = len(WAVES)
# AV DoubleRow pairs emitted after exp wave w (w-2 skew; trailing in-loop)
AV_SCHED = [[0, 1], [2, 3, 4], [5, 6], [7, 8], [9, 10, 11]]


def _at_col(jj):
    return 96 * jj


def _build_nc() -> bass.Bass:
    nc = bacc_mod.Bacc()

    xx_h = nc.declare_dram_parameter("xx", [NPAIR, 128, XXW], F8, isOutput=False)
    # ztp cols: [zt0 96][zt1 96] (zt1 rows 0:64)
    ztp_h = nc.declare_dram_parameter("ztp", [128, 192], BF, isOutput=False)
    # wq8 cols: [wq0 512][wq1 512] (wq1 rows 0:64), scaled by QSC, fp8
    wq8_h = nc.declare_dram_parameter("wq8", [128, 1024], F8, isOutput=False)
    bqt_h = nc.declare_dram_parameter("bqt", [128, 4], F32, isOutput=False)
    zbo_h = nc.declare_dram_parameter("zbo", [BPC, M, D], F32, isOutput=False)
    ident_h = nc.declare_dram_parameter("ident", [96, 96], BF, isOutput=False)
    # wod[p, 2u + i, d] = Wo[128*(2u+i) + p, d]  (fp8, DoubleRow layout)
    wod_h = nc.declare_dram_parameter("wod", [128, 768], F8, isOutput=False)
    out_h = nc.declare_dram_parameter("out", [BPC, M, D], F32, isOutput=True)

    # [12(t,m), 8(pair), 192(d)]: partition q=6t+m, free (pair, d)
    zbo_r = bass.AP(
        tensor=zbo_h.ap().tensor, offset=0,
        ap=[[D, 2 * M], [2 * M * D, NPAIR], [1, D]],
    )
    out_r = bass.AP(
        tensor=out_h.ap().tensor, offset=0,
        ap=[[D, 2 * M], [2 * M * D, NPAIR], [1, D]],
    )

    with tile.TileContext(nc) as tc, ExitStack() as ctx:
        const = ctx.enter_context(tc.tile_pool(name="const", bufs=1))
        xx_pool = ctx.enter_context(tc.tile_pool(name="xx", bufs=3))
        small = ctx.enter_context(tc.tile_pool(name="small", bufs=3))
        at_psum = ctx.enter_context(tc.tile_pool(name="at_ps", bufs=3, space="PSUM"))
        rs_psum = ctx.enter_context(tc.tile_pool(name="rs_ps", bufs=2, space="PSUM"))
        sm_psum = ctx.enter_context(tc.tile_pool(name="sm_ps", bufs=2, space="PSUM"))

        # ---------------- phase 0: constants / projections ----------------
        # Critical-path loads (qproj deps + first-wave xf piece) on the SP
        # HWDGE ring; everything else on the ACT ring.
        ztp = const.tile([128, 192], BF)
        nc.sync.dma_start(out=ztp, in_=ztp_h.ap())
        xf0a = const.tile([128, W0], F8)
        xx0_base = bass.AP(
            tensor=xx_h.ap().tensor, offset=0, ap=[[XXW, 128], [1, XXW]]
        )
        nc.sync.dma_start(out=xf0a, in_=xx0_base[:, 0:W0])
        wq8 = const.tile([128, 1024], F8)
        nc.sync.dma_start(out=wq8, in_=wq8_h.ap())

        bqt_sb = const.tile([128, 4], F32)
        nc.scalar.dma_start(out=bqt_sb, in_=bqt_h.ap())

        # warm the ACT exp table while DMAs stream (1.3us table load)
        warm = const.tile([1, 1], F32)
        nc.gpsimd.memset(warm, 0.0)
        warm2 = const.tile([1, 1], F32)
        nc.scalar.activation(
            out=warm2, in_=warm, func=mybir.ActivationFunctionType.Exp
        )
        ident_sb = const.tile([96, 96], BF)
        nc.scalar.dma_start(out=ident_sb, in_=ident_h.ap())
        wod_sb = const.tile([128, 768], F8)
        nc.scalar.dma_start(out=wod_sb, in_=wod_h.ap())
        zbo_sb = const.tile([12, NPAIR * D], F32)
        nc.scalar.dma_start(
            out=zbo_sb.rearrange("q (p d) -> q p d", p=NPAIR), in_=zbo_r
        )

        zt0 = ztp[:, 0:96]
        zt1 = ztp[0:64, 96:192]

        # Persistent per-pair attention-weight buffers (fp8): 25 chunk slots
        # of 96 cols each, contiguous so DoubleRow can pair adjacent chunks.
        ax_bufs = []
        for i in range(3):
            t = const.tile([128, NCHUNK * 96], F8, name=f"ax_buf{i}")
            ax_bufs.append(t)
        qT2_bufs = []
        for i in range(2):
            t = const.tile([128, 96], BF, name=f"qT2_buf{i}")
            nc.gpsimd.memset(t, 0.0)
            qT2_bufs.append(t)

        # q^T for all 16 local batches: qT_all[i, 6b+m] = (z @ Wq*QSC + ...)^T
        # chunk ii holds i in [128*ii, 128*ii+128)
        qT_all = const.tile([128, 4 * 96], BF)
        for ii in range(4):
            qp = at_psum.tile([128, 96], F32, tag="at", name=f"qp{ii}")
            nc.tensor.matmul(
                qp, lhsT=wq8[:, 128 * ii : 128 * ii + 128], rhs=zt0,
                start=True, stop=False,
            )
            nc.tensor.matmul(
                qp, lhsT=wq8[0:64, 512 + 128 * ii : 512 + 128 * ii + 128], rhs=zt1,
                start=False, stop=True,
            )
            nc.vector.tensor_scalar_add(
                out=qT_all[:, 96 * ii : 96 * ii + 96], in0=qp,
                scalar1=bqt_sb[:, ii : ii + 1],
            )

        # ---------------- per-pair main loop ----------------
        # column order inside a pair: hm2 = 48*b + u, u = 6*h + m.
        # Reference's q reshape is a FLAT view of [M, H*C], so the query row
        # for (h, m) is q_flat[(6h+m)//8, 64*((6h+m)%8) : +64].  With
        # u = 8*t + 2*ii + g: source chunk ii, partition half g, z-row t.
        qT_all_g = qT_all.rearrange("p (hh x) -> p hh x", hh=4)  # [128, 4, 96]

        xx_tiles = {}

        def emit_dma(p):
            xxt = xx_pool.tile([128, XXW], F8, tag="xx", name=f"xx{p}")
            src = bass.AP(
                tensor=xx_h.ap().tensor, offset=p * 128 * XXW,
                ap=[[XXW, 128], [1, XXW]],
            )
            if p == 0:
                nc.sync.dma_start(out=xxt[:, W0:XXW], in_=src[:, W0:XXW])
            else:
                nc.sync.dma_start(out=xxt, in_=src)
            xx_tiles[p] = xxt

        rsums = {}
        r2ns = {}
        rts = {}
        fcls = {}
        o2s = {}

        def emit_qk_wave(p, w, at):
            qT2 = qT2_bufs[p % 2]
            j0, nj = WAVES[w]
            for jj in range(nj):
                j = j0 + jj
                cw = 64 if j == NCHUNK - 1 else 128
                if p == 0 and w == 0:
                    lhs = xf0a[:, 128 * j : 128 * j + cw]
                else:
                    lhs = xx_tiles[p][:, 128 * j : 128 * j + cw]
                c0 = _at_col(jj)
                nc.tensor.matmul(
                    out=at[0:cw, c0 : c0 + 96], lhsT=lhs, rhs=qT2,
                    start=True, stop=True,
                )

        def emit_exp_wave(p, w, at):
            ax = ax_bufs[p % 3]
            ax_v = ax.rearrange("q (j c) -> q j c", j=NCHUNK)
            j0, nj = WAVES[w]
            last = w == NW - 1
            if w in (1, 3):
                # Schraudolph on DVE + bitcast-cast copy on GpSimd, freeing
                # the ACT engine (the exp bottleneck)
                sch = small.tile([128, 480], I32, tag="sch", name=f"sch{p}_{w}")
                nc.vector.tensor_scalar(
                    out=sch, in0=at, scalar1=SCH_A, scalar2=SCH_B,
                    op0=mybir.AluOpType.mult, op1=mybir.AluOpType.add,
                )
                nc.gpsimd.tensor_copy(
                    out=ax_v[:, j0 : j0 + nj, :],
                    in_=sch.bitcast(F32).rearrange("q (j c) -> q j c", j=nj),
                )
            elif not last:
                nc.scalar.activation(
                    out=ax_v[:, j0 : j0 + nj, :], in_=at,
                    func=mybir.ActivationFunctionType.Exp, scale=ESC,
                )
            else:
                nc.scalar.activation(
                    out=ax_v[:, j0 : j0 + 4, :], in_=at[:, 0:384],
                    func=mybir.ActivationFunctionType.Exp, scale=ESC,
                )
                nc.scalar.activation(
                    out=ax_v[0:64, j0 + 4, :], in_=at[0:64, 384:480],
                    func=mybir.ActivationFunctionType.Exp, scale=ESC,
                )

        def emit_av_dr(p, j2):
            # DoubleRow: contract chunks 2*j2 and 2*j2+1 in one matmul
            ax = ax_bufs[p % 3]
            xxt = xx_tiles[p]
            lhsT = ax[:, 192 * j2 : 192 * j2 + 192].rearrange(
                "q (i m) -> q i m", i=2
            )
            rhs_flat = xxt[:, HW + 2 * j2 * CSTRIDE : HW + (2 * j2 + 2) * CSTRIDE]
            rhs = rhs_flat.rearrange("q (i m) -> q i m", i=2)[:, :, 0:129]
            nc.tensor.matmul(
                out=rsums[p], lhsT=lhsT, rhs=rhs, perf_mode=DR,
                start=(j2 == 0), stop=False,
            )

        def emit_av_last(p):
            # chunk 24 (64-wide): plain fp8 matmul
            ax = ax_bufs[p % 3]
            xxt = xx_tiles[p]
            j = NCHUNK - 1
            nc.tensor.matmul(
                out=rsums[p],
                lhsT=ax[0:64, 96 * j : 96 * j + 96],
                rhs=xxt[0:64, HW + j * CSTRIDE : HW + j * CSTRIDE + 129],
                start=False, stop=True,
            )

        def emit_qt2(p):
            # block-diagonal qT2: [c2, hm2]; c2 = 64*b + c
            qT2 = qT2_bufs[p % 2]
            # col = 48*b + 8*t + 2*ii + g  ->  view [q, b, ii, t, g]
            qT2_v = qT2.rearrange("q (b t ii g) -> q b ii t g", b=2, t=6, ii=4)
            for b in range(2):
                for g in range(2):
                    dst = qT2_v[64 * b : 64 * b + 64, b, :, :, g]
                    src = qT_all_g[
                        64 * g : 64 * g + 64, :, 12 * p + 6 * b : 12 * p + 6 * b + 6
                    ]
                    nc.gpsimd.tensor_copy(out=dst, in_=src)

        def emit_norm(p):
            # softmax denominator is rsum[:, 128]; normalize
            rsum = rsums[p]
            inv = small.tile([96, 1], F32, tag="inv", name=f"inv{p}")
            nc.vector.reciprocal(out=inv, in_=rsum[:, 128:129])
            r2n = small.tile([96, 128], BF, tag="r2n", name=f"r2n{p}")
            nc.vector.tensor_scalar_mul(out=r2n, in0=rsum[:, 0:128], scalar1=inv)
            r2ns[p] = r2n

        def emit_rt(p):
            rt = sm_psum.tile([128, 96], BF, tag="sm", name=f"rt{p}")
            nc.tensor.transpose(rt, r2ns[p], ident_sb)
            rts[p] = rt

        def emit_fcl(p):
            # fc lhsT (fp8, DoubleRow layout [q, u, i, x16], kk = 2u + i):
            # fcl[64*hl + c, (16*(2u+i)) + 6*b + m]
            #   = rt[64*b + c, 48*b + 12*(2u+i) + 6*hl + m]   (h = 2*kk + hl)
            rt = rts[p]
            fcl = small.tile([128, 64], F8, tag="fcl", name=f"fcl{p}")
            fcl_g = fcl.rearrange("q (kk x) -> q kk x", kk=4)
            rt_v = rt.rearrange("q (b kk hl m) -> q b kk hl m", b=2, kk=4, hl=2)
            for hl in range(2):
                for b in range(2):
                    dst = fcl_g[64 * hl : 64 * hl + 64, :, 6 * b : 6 * b + 6]
                    src = rt_v[64 * b : 64 * b + 64, b, :, hl, :]
                    nc.vector.tensor_copy(out=dst, in_=src)
            fcls[p] = fcl

        def emit_fc(p):
            # o2[q, d] = sum_kk fcl[:, 16kk : 16kk+12].T @ Wo[128kk : +128, :]
            # as 2 DoubleRow matmuls (kk pairs (0,1) and (2,3))
            fcl = fcls[p]
            o2 = sm_psum.tile([16, D], F32, tag="sm", name=f"o2_{p}")
            for u in range(2):
                lhsT = fcl[:, 32 * u : 32 * u + 32].rearrange(
                    "q (i m) -> q i m", i=2
                )
                rhs = wod_sb[:, 384 * u : 384 * u + 384].rearrange(
                    "q (i m) -> q i m", i=2
                )
                nc.tensor.matmul(
                    out=o2, lhsT=lhsT, rhs=rhs, perf_mode=DR,
                    start=(u == 0), stop=(u == 1),
                )
            o2s[p] = o2

        def emit_out_add(p):
            nc.vector.tensor_add(
                out=out_all[:, D * p : D * (p + 1)], in0=o2s[p][0:12, :],
                in1=zbo_sb[:, D * p : D * (p + 1)],
            )
            out_rp = bass.AP(
                tensor=out_h.ap().tensor, offset=p * 2 * M * D,
                ap=[[D, 2 * M], [1, D]],
            )
            nc.sync.dma_start(out=out_rp, in_=out_all[:, D * p : D * (p + 1)])

        out_all = const.tile([12, NPAIR * D], F32)

        emit_dma(0)
        emit_dma(1)
        emit_qt2(0)
        for p in range(NPAIR):
            if p + 2 < NPAIR:
                emit_dma(p + 2)
            if p + 1 < NPAIR:
                emit_qt2(p + 1)
            rsums[p] = rs_psum.tile([96, 129], F32, tag="rs", name=f"rsum{p}")

            # software-pipelined waves: AV trails QK by two waves so exp()
            # latency is hidden; prior pair's tail (rt/fcl/fc/out) is
            # interleaved into this pair's wave stream.
            for w in range(NW):
                at = at_psum.tile([128, 5 * 96], F32, tag="at", name=f"at{p}_{w}")
                emit_qk_wave(p, w, at)
                emit_exp_wave(p, w, at)
                if w == 1 and p > 0:
                    emit_rt(p - 1)
                    emit_fcl(p - 1)
                if w == 2 and p > 0:
                    emit_fc(p - 1)
                    emit_out_add(p - 1)
                if w >= 2:
                    for j2 in AV_SCHED[w - 2]:
                        emit_av_dr(p, j2)
            for j2 in AV_SCHED[NW - 2]:
                emit_av_dr(p, j2)
            for j2 in AV_SCHED[NW - 1]:
                emit_av_dr(p, j2)
            emit_av_last(p)
            emit_norm(p)

        p = NPAIR - 1
        emit_rt(p)
        emit_fcl(p)
        emit_fc(p)
        emit_out_add(p)

    return nc


def get_nc() -> bass.Bass:
    if "nc" not in _CACHE:
        nc = _build_nc()
        # The PJRT exec path serializes nc.m as-is; run Bacc's legalization
        # (wait splitting, register allocation, ...) explicitly.
        nc.finalize()
        _CACHE["nc"] = nc
    return _CACHE["nc"]


def make_in_maps(x, z, Wq, bq, Wo, bo):
    """Host-side prep + sharding into per-core input maps."""
    x = np.asarray(x, dtype=np.float32)
    z = np.asarray(z, dtype=np.float32)
    Wq = np.asarray(Wq, dtype=np.float32)
    bq = np.asarray(bq, dtype=np.float32)
    Wo = np.asarray(Wo, dtype=np.float32)
    bo = np.asarray(bo, dtype=np.float32)

    x_f8 = x.astype(FP8)
    wq_s = (Wq * np.float32(QSC)).astype(FP8)  # QSC folded out in exp scale
    bqt = (bq * np.float32(QSC)).reshape(4, 128).T  # [128, 4], chunk ii col ii
    zbo = (z + bo[None, None, :]).astype(np.float32)
    ident = np.eye(96, dtype=BF16)
    # wod[p, 384u + 192i + d] = Wo[128*(2u+i) + p, d]
    wod = np.ascontiguousarray(
        Wo.astype(FP8).reshape(4, 128, D).transpose(1, 0, 2).reshape(128, 4 * D)
    )
    wq8 = np.zeros((128, 1024), dtype=FP8)
    wq8[:, 0:512] = wq_s[0:128]
    wq8[0:64, 512:1024] = wq_s[128:192]

    x_flat = x_f8.reshape(B, C, HW)

    in_maps = []
    for i in range(N_CORES):
        s = slice(i * BPC, (i + 1) * BPC)
        # zt[d, 6*b_local + m] = z[core_base + b_local, m, d]
        zt = z[s].reshape(BPC * M, D).T.astype(BF16)
        ztp = np.zeros((128, 192), dtype=BF16)
        ztp[:, 0:96] = zt[0:128]
        ztp[0:64, 96:192] = zt[128:192]

        xc = x_flat[s]  # [16, 64, 3136] fp8
        # xx = [xf | xt] per pair:
        # xf[c2, n] = x[2p + c2//64, c2%64, n]
        xf = xc.reshape(NPAIR, 2 * C, HW)
        # xt[r, 144j + c2] = x[2p + c2//64, c2%64, 128j + r]; col 128 = 1.0
        xp = xf.transpose(0, 2, 1)  # [8, 3136, 128]
        xpad = np.zeros((NPAIR, NCHUNK * 128, 128), dtype=FP8)
        xpad[:, 0:HW, :] = xp
        xj = xpad.reshape(NPAIR, NCHUNK, 128, 128).transpose(0, 2, 1, 3)
        xt = np.zeros((NPAIR, 128, NCHUNK, CSTRIDE), dtype=FP8)
        xt[:, :, :, 0:128] = xj
        xt[:, :, :, 128] = FP8(1.0)
        xx = np.concatenate([xf, xt.reshape(NPAIR, 128, XTW)], axis=2)

        in_maps.append(
            {
                "xx": np.ascontiguousarray(xx),
                "ztp": ztp,
                "wq8": wq8,
                "bqt": np.ascontiguousarray(bqt),
                "zbo": zbo[s],
                "ident": ident,
                "wod": wod,
            }
        )
    return in_maps


def kernel(**inputs) -> np.ndarray:
    nc = get_nc()
    in_maps = make_in_maps(
        inputs["x"], inputs["z"], inputs["Wq"], inputs["bq"],
        inputs["Wo"], inputs["bo"],
    )
    res = run_bass_kernel_spmd(nc, in_maps, list(range(N_CORES)))
    out = np.concatenate(
        [np.asarray(res.results[i]["out"]) for i in range(N_CORES)], axis=0
    )
    return out.astype(np.float32)


# revision 12
# speedup vs baseline: 1.5597x; 1.5597x over previous
"""Trainium2 Bass kernel for Mobile2Former cross-attention block.

Computation (per batch b):
    xf   = x[b].reshape(C, H*W)                      # [64, 3136] keys=values
    q    = (z[b] @ Wq + bq).reshape(heads, M, C)     # [8, 6, 64]
    attn = softmax(q @ xf * C**-0.5, axis=-1)        # [8, 6, 3136]
    res  = attn @ xf.T                               # [8, 6, 64]
    out  = res.transpose(1,0,2).reshape(M, -1) @ Wo + bo + z[b]

Strategy: data-parallel over B across 8 cores (16 batches/core), batches
processed in pairs (two batches stacked on the 128 SBUF partitions, C=64
each).  QK^T is computed directly in transposed layout (attn^T[n, hm]) by
using xf chunks (fp8) as the matmul stationary operand.  The AV matmul
consumes a host-pretransposed copy of x (fp8, with a ones column baked in
per chunk for the softmax denominator) as the moving operand, so no PE
transposes are needed.  AV and the output projection run in fp8 DoubleRow
mode (two 128-chunks contracted per matmul).  Softmax runs without max
subtraction (logits are O(1)); exp() applies the attention scale (folded
with an 8x fp8-range rescale of Wq) and writes fp8 attention weights
straight into a per-pair contiguous stationary buffer.
"""

import sys
from contextlib import ExitStack

import numpy as np

sys.path.insert(0, "/opt/trn_rl_repo")

import concourse.bass as bass
import concourse.tile as tile
from concourse import bacc as bacc_mod
from concourse import mybir
from concourse.bass_utils import run_bass_kernel_spmd

import ml_dtypes

BF16 = ml_dtypes.bfloat16
FP8 = ml_dtypes.float8_e4m3

N_CORES = 8
B, C, H, W = 128, 64, 56, 56
HW = H * W  # 3136
M, D = 6, 192
NH = 8
INNER = NH * C  # 512
BPC = B // N_CORES  # 16 batches per core
NPAIR = BPC // 2  # 8 pairs per core
NCHUNK = (HW + 127) // 128  # 25 (24 full + one 64-wide)
CSTRIDE = 144  # xt per-chunk col stride: 128 data + 1 ones + pad (16-mult)
XTW = NCHUNK * CSTRIDE  # 3600
XXW = HW + XTW  # 6736: packed [xf 3136 | xt 3600] per pair
W0 = 640  # first-wave xf piece loaded separately for fast start
QSC = 8.0  # Wq rescale so fp8 weights stay in normal range
ESC = float(C ** -0.5) / QSC  # exp() pre-scale: attn scale / QSC
# Schraudolph exp: exp(x*ESC) ~= bitcast_f32(int32(SCH_A*x + SCH_B))
SCH_A = (2.0 ** 23) / 0.6931471805599453 * ESC
SCH_B = 127.0 * 2 ** 23 - 366393.0  # min-RMS variant (~1.8% rel RMS)

F32 = mybir.dt.float32
I32 = mybir.dt.int32
BF = mybir.dt.bfloat16
F8 = mybir.dt.float8e4
DR = mybir.MatmulPerfMode.DoubleRow

_CACHE = {}

# waves of QK/exp chunks; last wave has the 64-wide chunk
WAVES = [(0, 5), (5, 5), (10, 5), (15, 5), (20, 5)]
NW = len(WAVES)
# AV DoubleRow pairs emitted after exp wave w (w-2 skew; trailing in-loop)
AV_SCHED = [[0, 1], [2, 3, 4], [5, 6], [7, 8], [9, 10, 11]]


def _at_col(jj):
    return 96 * jj


def _build_nc() -> bass.Bass:
    nc = bacc_mod.Bacc()

    xx_h = nc.declare_dram_parameter("xx", [NPAIR, 128, XXW], F8, isOutput=False)
    # ztp cols: [zt0 96][zt1 96] (zt1 rows 0:64)
    ztp_h = nc.declare_dram_parameter("ztp", [128, 192], BF, isOutput=False)
    # wq8 cols: [wq0 512][wq1 512] (wq1 rows 0:64), scaled by QSC, fp8
    wq8_h = nc.declare_dram_parameter("wq8", [128, 1024], F8, isOutput=False)
    bqt_h = nc.declare_dram_parameter("bqt", [128, 4], F32, isOutput=False)
    zbo_h = nc.declare_dram_parameter("zbo", [BPC, M, D], F32, isOutput=False)
    ident_h = nc.declare_dram_parameter("ident", [96, 96], BF, isOutput=False)
    # wod[p, 2u + i, d] = Wo[128*(2u+i) + p, d]  (fp8, DoubleRow layout)
    wod_h = nc.declare_dram_parameter("wod", [128, 768], F8, isOutput=False)
    out_h = nc.declare_dram_parameter("out", [BPC, M, D], F32, isOutput=True)

    # [12(t,m), 8(pair), 192(d)]: partition q=6t+m, free (pair, d)
    zbo_r = bass.AP(
        tensor=zbo_h.ap().tensor, offset=0,
        ap=[[D, 2 * M], [2 * M * D, NPAIR], [1, D]],
    )
    out_r = bass.AP(
        tensor=out_h.ap().tensor, offset=0,
        ap=[[D, 2 * M], [2 * M * D, NPAIR], [1, D]],
    )

    with tile.TileContext(nc) as tc, ExitStack() as ctx:
        const = ctx.enter_context(tc.tile_pool(name="const", bufs=1))
        xx_pool = ctx.enter_context(tc.tile_pool(name="xx", bufs=3))
        small = ctx.enter_context(tc.tile_pool(name="small", bufs=3))
        at_psum = ctx.enter_context(tc.tile_pool(name="at_ps", bufs=3, space="PSUM"))
        rs_psum = ctx.enter_context(tc.tile_pool(name="rs_ps", bufs=2, space="PSUM"))
        sm_psum = ctx.enter_context(tc.tile_pool(name="sm_ps", bufs=2, space="PSUM"))

        # ---------------- phase 0: constants / projections ----------------
        # Critical-path loads (qproj deps + first-wave xf piece) on the SP
        # HWDGE ring; everything else on the ACT ring.
        ztp = const.tile([128, 192], BF)
        nc.sync.dma_start(out=ztp, in_=ztp_h.ap())
        xf0a = const.tile([128, W0], F8)
        xx0_base = bass.AP(
            tensor=xx_h.ap().tensor, offset=0, ap=[[XXW, 128], [1, XXW]]
        )
        nc.sync.dma_start(out=xf0a, in_=xx0_base[:, 0:W0])
        wq8 = const.tile([128, 1024], F8)
        nc.sync.dma_start(out=wq8, in_=wq8_h.ap())

        bqt_sb = const.tile([128, 4], F32)
        nc.scalar.dma_start(out=bqt_sb, in_=bqt_h.ap())

        # warm the ACT exp table while DMAs stream (1.3us table load)
        warm = const.tile([1, 1], F32)
        nc.gpsimd.memset(warm, 0.0)
        warm2 = const.tile([1, 1], F32)
        nc.scalar.activation(
            out=warm2, in_=warm, func=mybir.ActivationFunctionType.Exp
        )
        ident_sb = const.tile([96, 96], BF)
        nc.scalar.dma_start(out=ident_sb, in_=ident_h.ap())
        wod_sb = const.tile([128, 768], F8)
        nc.scalar.dma_start(out=wod_sb, in_=wod_h.ap())
        zbo_sb = const.tile([12, NPAIR * D], F32)
        nc.scalar.dma_start(
            out=zbo_sb.rearrange("q (p d) -> q p d", p=NPAIR), in_=zbo_r
        )

        zt0 = ztp[:, 0:96]
        zt1 = ztp[0:64, 96:192]

        # Persistent per-pair attention-weight buffers (fp8): 25 chunk slots
        # of 96 cols each, contiguous so DoubleRow can pair adjacent chunks.
        ax_bufs = []
        for i in range(3):
            t = const.tile([128, NCHUNK * 96], F8, name=f"ax_buf{i}")
            ax_bufs.append(t)
        qT2_bufs = []
        for i in range(2):
            t = const.tile([128, 96], BF, name=f"qT2_buf{i}")
            nc.gpsimd.memset(t, 0.0)
            qT2_bufs.append(t)

        # q^T for all 16 local batches: qT_all[i, 6b+m] = (z @ Wq*QSC + ...)^T
        # chunk ii holds i in [128*ii, 128*ii+128)
        qT_all = const.tile([128, 4 * 96], BF)
        for ii in range(4):
            qp = at_psum.tile([128, 96], F32, tag="at", name=f"qp{ii}")
            nc.tensor.matmul(
                qp, lhsT=wq8[:, 128 * ii : 128 * ii + 128], rhs=zt0,
                start=True, stop=False,
            )
            nc.tensor.matmul(
                qp, lhsT=wq8[0:64, 512 + 128 * ii : 512 + 128 * ii + 128], rhs=zt1,
                start=False, stop=True,
            )
            nc.vector.tensor_scalar_add(
                out=qT_all[:, 96 * ii : 96 * ii + 96], in0=qp,
                scalar1=bqt_sb[:, ii : ii + 1],
            )

        # ---------------- per-pair main loop ----------------
        # column order inside a pair: hm2 = 48*b + u, u = 6*h + m.
        # Reference's q reshape is a FLAT view of [M, H*C], so the query row
        # for (h, m) is q_flat[(6h+m)//8, 64*((6h+m)%8) : +64].  With
        # u = 8*t + 2*ii + g: source chunk ii, partition half g, z-row t.
        qT_all_g = qT_all.rearrange("p (hh x) -> p hh x", hh=4)  # [128, 4, 96]

        xx_tiles = {}

        def emit_dma(p):
            xxt = xx_pool.tile([128, XXW], F8, tag="xx", name=f"xx{p}")
            src = bass.AP(
                tensor=xx_h.ap().tensor, offset=p * 128 * XXW,
                ap=[[XXW, 128], [1, XXW]],
            )
            if p == 0:
                nc.sync.dma_start(out=xxt[:, W0:XXW], in_=src[:, W0:XXW])
            else:
                nc.sync.dma_start(out=xxt, in_=src)
            xx_tiles[p] = xxt

        rsums = {}
        r2ns = {}
        rts = {}
        fcls = {}
        o2s = {}

        def emit_qk_wave(p, w, at):
            qT2 = qT2_bufs[p % 2]
            j0, nj = WAVES[w]
            for jj in range(nj):
                j = j0 + jj
                cw = 64 if j == NCHUNK - 1 else 128
                if p == 0 and w == 0:
                    lhs = xf0a[:, 128 * j : 128 * j + cw]
                else:
                    lhs = xx_tiles[p][:, 128 * j : 128 * j + cw]
                c0 = _at_col(jj)
                nc.tensor.matmul(
                    out=at[0:cw, c0 : c0 + 96], lhsT=lhs, rhs=qT2,
                    start=True, stop=True,
                )

        def emit_exp_wave(p, w, at):
            ax = ax_bufs[p % 3]
            ax_v = ax.rearrange("q (j c) -> q j c", j=NCHUNK)
            j0, nj = WAVES[w]
            last = w == NW - 1
            if w in (1, 3):
                # Schraudolph on DVE + bitcast-cast copy on GpSimd, freeing
                # the ACT engine (the exp bottleneck)
                sch = small.tile([128, 480], I32, tag="sch", name=f"sch{p}_{w}")
                nc.vector.tensor_scalar(
                    out=sch, in0=at, scalar1=SCH_A, scalar2=SCH_B,
                    op0=mybir.AluOpType.mult, op1=mybir.AluOpType.add,
                )
                nc.vector.tensor_copy(
                    out=ax_v[:, j0 : j0 + nj, :],
                    in_=sch.bitcast(F32).rearrange("q (j c) -> q j c", j=nj),
                )
            elif not last:
                nc.scalar.activation(
                    out=ax_v[:, j0 : j0 + nj, :], in_=at,
                    func=mybir.ActivationFunctionType.Exp, scale=ESC,
                )
            else:
                nc.scalar.activation(
                    out=ax_v[:, j0 : j0 + 4, :], in_=at[:, 0:384],
                    func=mybir.ActivationFunctionType.Exp, scale=ESC,
                )
                nc.scalar.activation(
                    out=ax_v[0:64, j0 + 4, :], in_=at[0:64, 384:480],
                    func=mybir.ActivationFunctionType.Exp, scale=ESC,
                )

        def emit_av_dr(p, j2):
            # DoubleRow: contract chunks 2*j2 and 2*j2+1 in one matmul
            ax = ax_bufs[p % 3]
            xxt = xx_tiles[p]
            lhsT = ax[:, 192 * j2 : 192 * j2 + 192].rearrange(
                "q (i m) -> q i m", i=2
            )
            rhs_flat = xxt[:, HW + 2 * j2 * CSTRIDE : HW + (2 * j2 + 2) * CSTRIDE]
            rhs = rhs_flat.rearrange("q (i m) -> q i m", i=2)[:, :, 0:129]
            nc.tensor.matmul(
                out=rsums[p], lhsT=lhsT, rhs=rhs, perf_mode=DR,
                start=(j2 == 0), stop=False,
            )

        def emit_av_last(p):
            # chunk 24 (64-wide): plain fp8 matmul
            ax = ax_bufs[p % 3]
            xxt = xx_tiles[p]
            j = NCHUNK - 1
            nc.tensor.matmul(
                out=rsums[p],
                lhsT=ax[0:64, 96 * j : 96 * j + 96],
                rhs=xxt[0:64, HW + j * CSTRIDE : HW + j * CSTRIDE + 129],
                start=False, stop=True,
            )

        def emit_qt2(p):
            # block-diagonal qT2: [c2, hm2]; c2 = 64*b + c
            qT2 = qT2_bufs[p % 2]
            # col = 48*b + 8*t + 2*ii + g  ->  view [q, b, ii, t, g]
            qT2_v = qT2.rearrange("q (b t ii g) -> q b ii t g", b=2, t=6, ii=4)
            for b in range(2):
                for g in range(2):
                    dst = qT2_v[64 * b : 64 * b + 64, b, :, :, g]
                    src = qT_all_g[
                        64 * g : 64 * g + 64, :, 12 * p + 6 * b : 12 * p + 6 * b + 6
                    ]
                    nc.gpsimd.tensor_copy(out=dst, in_=src)

        def emit_norm(p):
            # softmax denominator is rsum[:, 128]; normalize
            rsum = rsums[p]
            inv = small.tile([96, 1], F32, tag="inv", name=f"inv{p}")
            nc.vector.reciprocal(out=inv, in_=rsum[:, 128:129])
            r2n = small.tile([96, 128], BF, tag="r2n", name=f"r2n{p}")
            nc.vector.tensor_scalar_mul(out=r2n, in0=rsum[:, 0:128], scalar1=inv)
            r2ns[p] = r2n

        def emit_rt(p):
            rt = sm_psum.tile([128, 96], BF, tag="sm", name=f"rt{p}")
            nc.tensor.transpose(rt, r2ns[p], ident_sb)
            rts[p] = rt

        def emit_fcl(p):
            # fc lhsT (fp8, DoubleRow layout [q, u, i, x16], kk = 2u + i):
            # fcl[64*hl + c, (16*(2u+i)) + 6*b + m]
            #   = rt[64*b + c, 48*b + 12*(2u+i) + 6*hl + m]   (h = 2*kk + hl)
            rt = rts[p]
            fcl = small.tile([128, 64], F8, tag="fcl", name=f"fcl{p}")
            fcl_g = fcl.rearrange("q (kk x) -> q kk x", kk=4)
            rt_v = rt.rearrange("q (b kk hl m) -> q b kk hl m", b=2, kk=4, hl=2)
            for hl in range(2):
                for b in range(2):
                    dst = fcl_g[64 * hl : 64 * hl + 64, :, 6 * b : 6 * b + 6]
                    src = rt_v[64 * b : 64 * b + 64, b, :, hl, :]
                    nc.vector.tensor_copy(out=dst, in_=src)
            fcls[p] = fcl

        def emit_fc(p):
            # o2[q, d] = sum_kk fcl[:, 16kk : 16kk+12].T @ Wo[128kk : +128, :]
            # as 2 DoubleRow matmuls (kk pairs (0,1) and (2,3))
            fcl = fcls[p]
            o2 = sm_psum.tile([16, D], F32, tag="sm", name=f"o2_{p}")
            for u in range(2):
                lhsT = fcl[:, 32 * u : 32 * u + 32].rearrange(
                    "q (i m) -> q i m", i=2
                )
                rhs = wod_sb[:, 384 * u : 384 * u + 384].rearrange(
                    "q (i m) -> q i m", i=2
                )
                nc.tensor.matmul(
                    out=o2, lhsT=lhsT, rhs=rhs, perf_mode=DR,
                    start=(u == 0), stop=(u == 1),
                )
            o2s[p] = o2

        def emit_out_add(p):
            nc.vector.tensor_add(
                out=out_all[:, D * p : D * (p + 1)], in0=o2s[p][0:12, :],
                in1=zbo_sb[:, D * p : D * (p + 1)],
            )
            out_rp = bass.AP(
                tensor=out_h.ap().tensor, offset=p * 2 * M * D,
                ap=[[D, 2 * M], [1, D]],
            )
            nc.sync.dma_start(out=out_rp, in_=out_all[:, D * p : D * (p + 1)])

        out_all = const.tile([12, NPAIR * D], F32)

        emit_dma(0)
        emit_dma(1)
        emit_qt2(0)
        for p in range(NPAIR):
            if p + 2 < NPAIR:
                emit_dma(p + 2)
            if p + 1 < NPAIR:
                emit_qt2(p + 1)
            rsums[p] = rs_psum.tile([96, 129], F32, tag="rs", name=f"rsum{p}")

            # software-pipelined waves: AV trails QK by two waves so exp()
            # latency is hidden; prior pair's tail (rt/fcl/fc/out) is
            # interleaved into this pair's wave stream.
            for w in range(NW):
                at = at_psum.tile([128, 5 * 96], F32, tag="at", name=f"at{p}_{w}")
                emit_qk_wave(p, w, at)
                emit_exp_wave(p, w, at)
                if w == 1 and p > 0:
                    emit_rt(p - 1)
                    emit_fcl(p - 1)
                if w == 2 and p > 0:
                    emit_fc(p - 1)
                    emit_out_add(p - 1)
                if w >= 2:
                    for j2 in AV_SCHED[w - 2]:
                        emit_av_dr(p, j2)
            for j2 in AV_SCHED[NW - 2]:
                emit_av_dr(p, j2)
            for j2 in AV_SCHED[NW - 1]:
                emit_av_dr(p, j2)
            emit_av_last(p)
            emit_norm(p)

        p = NPAIR - 1
        emit_rt(p)
        emit_fcl(p)
        emit_fc(p)
        emit_out_add(p)

    return nc


def get_nc() -> bass.Bass:
    if "nc" not in _CACHE:
        nc = _build_nc()
        # The PJRT exec path serializes nc.m as-is; run Bacc's legalization
        # (wait splitting, register allocation, ...) explicitly.
        nc.finalize()
        _CACHE["nc"] = nc
    return _CACHE["nc"]


def make_in_maps(x, z, Wq, bq, Wo, bo):
    """Host-side prep + sharding into per-core input maps."""
    x = np.asarray(x, dtype=np.float32)
    z = np.asarray(z, dtype=np.float32)
    Wq = np.asarray(Wq, dtype=np.float32)
    bq = np.asarray(bq, dtype=np.float32)
    Wo = np.asarray(Wo, dtype=np.float32)
    bo = np.asarray(bo, dtype=np.float32)

    x_f8 = x.astype(FP8)
    wq_s = (Wq * np.float32(QSC)).astype(FP8)  # QSC folded out in exp scale
    bqt = (bq * np.float32(QSC)).reshape(4, 128).T  # [128, 4], chunk ii col ii
    zbo = (z + bo[None, None, :]).astype(np.float32)
    ident = np.eye(96, dtype=BF16)
    # wod[p, 384u + 192i + d] = Wo[128*(2u+i) + p, d]
    wod = np.ascontiguousarray(
        Wo.astype(FP8).reshape(4, 128, D).transpose(1, 0, 2).reshape(128, 4 * D)
    )
    wq8 = np.zeros((128, 1024), dtype=FP8)
    wq8[:, 0:512] = wq_s[0:128]
    wq8[0:64, 512:1024] = wq_s[128:192]

    x_flat = x_f8.reshape(B, C, HW)

    in_maps = []
    for i in range(N_CORES):
        s = slice(i * BPC, (i + 1) * BPC)
        # zt[d, 6*b_local + m] = z[core_base + b_local, m, d]
        zt = z[s].reshape(BPC * M, D).T.astype(BF16)
        ztp = np.zeros((128, 192), dtype=BF16)
        ztp[:, 0:96] = zt[0:128]
        ztp[0:64, 96:192] = zt[128:192]

        xc = x_flat[s]  # [16, 64, 3136] fp8
        # xx = [xf | xt] per pair:
        # xf[c2, n] = x[2p + c2//64, c2%64, n]
        xf = xc.reshape(NPAIR, 2 * C, HW)
        # xt[r, 144j + c2] = x[2p + c2//64, c2%64, 128j + r]; col 128 = 1.0
        xp = xf.transpose(0, 2, 1)  # [8, 3136, 128]
        xpad = np.zeros((NPAIR, NCHUNK * 128, 128), dtype=FP8)
        xpad[:, 0:HW, :] = xp
        xj = xpad.reshape(NPAIR, NCHUNK, 128, 128).transpose(0, 2, 1, 3)
        xt = np.zeros((NPAIR, 128, NCHUNK, CSTRIDE), dtype=FP8)
        xt[:, :, :, 0:128] = xj
        xt[:, :, :, 128] = FP8(1.0)
        xx = np.concatenate([xf, xt.reshape(NPAIR, 128, XTW)], axis=2)

        in_maps.append(
            {
                "xx": np.ascontiguousarray(xx),
                "ztp": ztp,
                "wq8": wq8,
                "bqt": np.ascontiguousarray(bqt),
                "zbo": zbo[s],
                "ident": ident,
                "wod": wod,
            }
        )
    return in_maps


def kernel(**inputs) -> np.ndarray:
    nc = get_nc()
    in_maps = make_in_maps(
        inputs["x"], inputs["z"], inputs["Wq"], inputs["bq"],
        inputs["Wo"], inputs["bo"],
    )
    res = run_bass_kernel_spmd(nc, in_maps, list(range(N_CORES)))
    out = np.concatenate(
        [np.asarray(res.results[i]["out"]) for i in range(N_CORES)], axis=0
    )
    return out.astype(np.float32)


# revision 13
# speedup vs baseline: 1.7689x; 1.1342x over previous
"""Trainium2 Bass kernel for Mobile2Former cross-attention block.

Computation (per batch b):
    xf   = x[b].reshape(C, H*W)                      # [64, 3136] keys=values
    q    = (z[b] @ Wq + bq).reshape(heads, M, C)     # [8, 6, 64]
    attn = softmax(q @ xf * C**-0.5, axis=-1)        # [8, 6, 3136]
    res  = attn @ xf.T                               # [8, 6, 64]
    out  = res.transpose(1,0,2).reshape(M, -1) @ Wo + bo + z[b]

Strategy: data-parallel over B across 8 cores (16 batches/core), batches
processed in pairs (two batches stacked on the 128 SBUF partitions, C=64
each).  QK^T is computed directly in transposed layout (attn^T[n, hm]) by
using xf chunks (fp8) as the matmul stationary operand.  The AV matmul
consumes a host-pretransposed copy of x (fp8, with a ones column baked in
per chunk for the softmax denominator) as the moving operand, so no PE
transposes are needed.  AV and the output projection run in fp8 DoubleRow
mode (two 128-chunks contracted per matmul).  Softmax runs without max
subtraction (logits are O(1)); exp() applies the attention scale (folded
with an 8x fp8-range rescale of Wq) and writes fp8 attention weights
straight into a per-pair contiguous stationary buffer.
"""

import sys
from contextlib import ExitStack

import numpy as np

sys.path.insert(0, "/opt/trn_rl_repo")

import concourse.bass as bass
import concourse.tile as tile
from concourse import bacc as bacc_mod
from concourse import mybir
from concourse.bass_utils import run_bass_kernel_spmd

import ml_dtypes

BF16 = ml_dtypes.bfloat16
FP8 = ml_dtypes.float8_e4m3

N_CORES = 8
B, C, H, W = 128, 64, 56, 56
HW = H * W  # 3136
M, D = 6, 192
NH = 8
INNER = NH * C  # 512
BPC = B // N_CORES  # 16 batches per core
NPAIR = BPC // 2  # 8 pairs per core
NCHUNK = (HW + 127) // 128  # 25 (24 full + one 64-wide)
CSTRIDE = 144  # xt per-chunk col stride: 128 data + 1 ones + pad (16-mult)
XTW = NCHUNK * CSTRIDE  # 3600
XXW = HW + XTW  # 6736: packed [xf 3136 | xt 3600] per pair
W0 = 640  # first-wave xf piece loaded separately for fast start
QSC = 8.0  # Wq rescale so fp8 weights stay in normal range
ESC = float(C ** -0.5) / QSC  # exp() pre-scale: attn scale / QSC
# Schraudolph exp: exp(x*ESC) ~= bitcast_f32(int32(SCH_A*x + SCH_B))
SCH_A = (2.0 ** 23) / 0.6931471805599453 * ESC
SCH_B = 127.0 * 2 ** 23 - 366393.0  # min-RMS variant (~1.8% rel RMS)

F32 = mybir.dt.float32
I32 = mybir.dt.int32
BF = mybir.dt.bfloat16
F8 = mybir.dt.float8e4
DR = mybir.MatmulPerfMode.DoubleRow

_CACHE = {}

# waves of QK/exp chunks; last wave has the 64-wide chunk
WAVES = [(0, 5), (5, 5), (10, 5), (15, 5), (20, 5)]
NW = len(WAVES)
# AV DoubleRow pairs emitted after exp wave w (w-2 skew; trailing in-loop)
AV_SCHED = [[0, 1], [2, 3, 4], [5, 6], [7, 8], [9, 10, 11]]


def _at_col(jj):
    return 96 * jj


def _build_nc() -> bass.Bass:
    nc = bacc_mod.Bacc()

    xx_h = nc.declare_dram_parameter("xx", [NPAIR, 128, XXW], F8, isOutput=False)
    # qpk cols: [zt0 96][zt1 96][wq0 512][wq1 512] (zt1/wq1 rows 0:64); fp8
    qpk_h = nc.declare_dram_parameter("qpk", [128, 1216], F8, isOutput=False)
    bqt_h = nc.declare_dram_parameter("bqt", [128, 4], F32, isOutput=False)
    zbo_h = nc.declare_dram_parameter("zbo", [BPC, M, D], F32, isOutput=False)
    ident_h = nc.declare_dram_parameter("ident", [96, 96], BF, isOutput=False)
    # wod[p, 2u + i, d] = Wo[128*(2u+i) + p, d]  (fp8, DoubleRow layout)
    wod_h = nc.declare_dram_parameter("wod", [128, 768], F8, isOutput=False)
    out_h = nc.declare_dram_parameter("out", [BPC, M, D], F32, isOutput=True)

    # [12(t,m), 8(pair), 192(d)]: partition q=6t+m, free (pair, d)
    zbo_r = bass.AP(
        tensor=zbo_h.ap().tensor, offset=0,
        ap=[[D, 2 * M], [2 * M * D, NPAIR], [1, D]],
    )
    out_r = bass.AP(
        tensor=out_h.ap().tensor, offset=0,
        ap=[[D, 2 * M], [2 * M * D, NPAIR], [1, D]],
    )

    with tile.TileContext(nc) as tc, ExitStack() as ctx:
        const = ctx.enter_context(tc.tile_pool(name="const", bufs=1))
        xx_pool = ctx.enter_context(tc.tile_pool(name="xx", bufs=4))
        small = ctx.enter_context(tc.tile_pool(name="small", bufs=3))
        at_psum = ctx.enter_context(tc.tile_pool(name="at_ps", bufs=3, space="PSUM"))
        rs_psum = ctx.enter_context(tc.tile_pool(name="rs_ps", bufs=2, space="PSUM"))
        sm_psum = ctx.enter_context(tc.tile_pool(name="sm_ps", bufs=2, space="PSUM"))

        # ---------------- phase 0: constants / projections ----------------
        # Critical-path loads (qproj deps + first-wave xf piece) on the SP
        # HWDGE ring; everything else on the ACT ring.
        qpk = const.tile([128, 1216], F8)
        nc.sync.dma_start(out=qpk, in_=qpk_h.ap())
        xf0a = const.tile([128, W0], F8)
        xx0_base = bass.AP(
            tensor=xx_h.ap().tensor, offset=0, ap=[[XXW, 128], [1, XXW]]
        )
        nc.sync.dma_start(out=xf0a, in_=xx0_base[:, 0:W0])

        bqt_sb = const.tile([128, 4], F32)
        nc.scalar.dma_start(out=bqt_sb, in_=bqt_h.ap())

        # warm the ACT exp table while DMAs stream (1.3us table load)
        warm = const.tile([1, 1], F32)
        nc.gpsimd.memset(warm, 0.0)
        warm2 = const.tile([1, 1], F32)
        nc.scalar.activation(
            out=warm2, in_=warm, func=mybir.ActivationFunctionType.Exp
        )
        ident_sb = const.tile([96, 96], BF)
        nc.scalar.dma_start(out=ident_sb, in_=ident_h.ap())
        wod_sb = const.tile([128, 768], F8)
        nc.scalar.dma_start(out=wod_sb, in_=wod_h.ap())
        zbo_sb = const.tile([12, NPAIR * D], F32)
        nc.scalar.dma_start(
            out=zbo_sb.rearrange("q (p d) -> q p d", p=NPAIR), in_=zbo_r
        )

        zt0 = qpk[:, 0:96]
        zt1 = qpk[0:64, 96:192]

        # Persistent per-pair attention-weight buffers (fp8): 25 chunk slots
        # of 96 cols each, contiguous so DoubleRow can pair adjacent chunks.
        ax_bufs = []
        for i in range(3):
            t = const.tile([128, NCHUNK * 96], F8, name=f"ax_buf{i}")
            ax_bufs.append(t)
        qT2_bufs = []
        for i in range(2):
            t = const.tile([128, 96], BF, name=f"qT2_buf{i}")
            nc.gpsimd.memset(t, 0.0)
            qT2_bufs.append(t)

        # q^T for all 16 local batches: qT_all[i, 6b+m] = (z @ Wq*QSC + ...)^T
        # chunk ii holds i in [128*ii, 128*ii+128)
        qT_all = const.tile([128, 4 * 96], BF)
        for ii in range(4):
            qp = at_psum.tile([128, 96], F32, tag="at", name=f"qp{ii}")
            nc.tensor.matmul(
                qp, lhsT=qpk[:, 192 + 128 * ii : 192 + 128 * ii + 128], rhs=zt0,
                start=True, stop=False,
            )
            nc.tensor.matmul(
                qp, lhsT=qpk[0:64, 704 + 128 * ii : 704 + 128 * ii + 128], rhs=zt1,
                start=False, stop=True,
            )
            nc.vector.tensor_scalar_add(
                out=qT_all[:, 96 * ii : 96 * ii + 96], in0=qp,
                scalar1=bqt_sb[:, ii : ii + 1],
            )

        # ---------------- per-pair main loop ----------------
        # column order inside a pair: hm2 = 48*b + u, u = 6*h + m.
        # Reference's q reshape is a FLAT view of [M, H*C], so the query row
        # for (h, m) is q_flat[(6h+m)//8, 64*((6h+m)%8) : +64].  With
        # u = 8*t + 2*ii + g: source chunk ii, partition half g, z-row t.
        qT_all_g = qT_all.rearrange("p (hh x) -> p hh x", hh=4)  # [128, 4, 96]

        xx_tiles = {}

        def emit_dma(p):
            xxt = xx_pool.tile([128, XXW], F8, tag="xx", name=f"xx{p}")
            src = bass.AP(
                tensor=xx_h.ap().tensor, offset=p * 128 * XXW,
                ap=[[XXW, 128], [1, XXW]],
            )
            if p == 0:
                nc.sync.dma_start(out=xxt[:, W0:XXW], in_=src[:, W0:XXW])
            else:
                nc.sync.dma_start(out=xxt, in_=src)
            xx_tiles[p] = xxt

        rsums = {}
        r2ns = {}
        rts = {}
        fcls = {}
        o2s = {}

        def emit_qk_wave(p, w, at):
            qT2 = qT2_bufs[p % 2]
            j0, nj = WAVES[w]
            for jj in range(nj):
                j = j0 + jj
                cw = 64 if j == NCHUNK - 1 else 128
                if p == 0 and w == 0:
                    lhs = xf0a[:, 128 * j : 128 * j + cw]
                else:
                    lhs = xx_tiles[p][:, 128 * j : 128 * j + cw]
                c0 = _at_col(jj)
                nc.tensor.matmul(
                    out=at[0:cw, c0 : c0 + 96], lhsT=lhs, rhs=qT2,
                    start=True, stop=True,
                )

        def emit_exp_wave(p, w, at):
            ax = ax_bufs[p % 3]
            ax_v = ax.rearrange("q (j c) -> q j c", j=NCHUNK)
            j0, nj = WAVES[w]
            last = w == NW - 1
            if w == 2:
                # Schraudolph on DVE + bitcast-cast copy on GpSimd, freeing
                # the ACT engine (the exp bottleneck)
                sch = small.tile([128, 480], I32, tag="sch", name=f"sch{p}_{w}")
                nc.vector.tensor_scalar(
                    out=sch, in0=at, scalar1=SCH_A, scalar2=SCH_B,
                    op0=mybir.AluOpType.mult, op1=mybir.AluOpType.add,
                )
                nc.vector.tensor_copy(
                    out=ax_v[:, j0 : j0 + nj, :],
                    in_=sch.bitcast(F32).rearrange("q (j c) -> q j c", j=nj),
                )
            elif not last:
                nc.scalar.activation(
                    out=ax_v[:, j0 : j0 + nj, :], in_=at,
                    func=mybir.ActivationFunctionType.Exp, scale=ESC,
                )
            else:
                nc.scalar.activation(
                    out=ax_v[:, j0 : j0 + 4, :], in_=at[:, 0:384],
                    func=mybir.ActivationFunctionType.Exp, scale=ESC,
                )
                nc.scalar.activation(
                    out=ax_v[0:64, j0 + 4, :], in_=at[0:64, 384:480],
                    func=mybir.ActivationFunctionType.Exp, scale=ESC,
                )

        def emit_av_dr(p, j2):
            # DoubleRow: contract chunks 2*j2 and 2*j2+1 in one matmul
            ax = ax_bufs[p % 3]
            xxt = xx_tiles[p]
            lhsT = ax[:, 192 * j2 : 192 * j2 + 192].rearrange(
                "q (i m) -> q i m", i=2
            )
            rhs_flat = xxt[:, HW + 2 * j2 * CSTRIDE : HW + (2 * j2 + 2) * CSTRIDE]
            rhs = rhs_flat.rearrange("q (i m) -> q i m", i=2)[:, :, 0:129]
            nc.tensor.matmul(
                out=rsums[p], lhsT=lhsT, rhs=rhs, perf_mode=DR,
                start=(j2 == 0), stop=False,
            )

        def emit_av_last(p):
            # chunk 24 (64-wide): plain fp8 matmul
            ax = ax_bufs[p % 3]
            xxt = xx_tiles[p]
            j = NCHUNK - 1
            nc.tensor.matmul(
                out=rsums[p],
                lhsT=ax[0:64, 96 * j : 96 * j + 96],
                rhs=xxt[0:64, HW + j * CSTRIDE : HW + j * CSTRIDE + 129],
                start=False, stop=True,
            )

        def emit_qt2(p):
            # block-diagonal qT2: [c2, hm2]; c2 = 64*b + c
            qT2 = qT2_bufs[p % 2]
            # col = 48*b + 8*t + 2*ii + g  ->  view [q, b, ii, t, g]
            qT2_v = qT2.rearrange("q (b t ii g) -> q b ii t g", b=2, t=6, ii=4)
            for b in range(2):
                for g in range(2):
                    dst = qT2_v[64 * b : 64 * b + 64, b, :, :, g]
                    src = qT_all_g[
                        64 * g : 64 * g + 64, :, 12 * p + 6 * b : 12 * p + 6 * b + 6
                    ]
                    nc.gpsimd.tensor_copy(out=dst, in_=src)

        def emit_norm(p):
            # softmax denominator is rsum[:, 128]; normalize
            rsum = rsums[p]
            inv = small.tile([96, 1], F32, tag="inv", name=f"inv{p}")
            nc.vector.reciprocal(out=inv, in_=rsum[:, 128:129])
            r2n = small.tile([96, 128], BF, tag="r2n", name=f"r2n{p}")
            nc.vector.tensor_scalar_mul(out=r2n, in0=rsum[:, 0:128], scalar1=inv)
            r2ns[p] = r2n

        def emit_rt(p):
            rt = sm_psum.tile([128, 96], BF, tag="sm", name=f"rt{p}")
            nc.tensor.transpose(rt, r2ns[p], ident_sb)
            rts[p] = rt

        def emit_fcl(p):
            # fc lhsT (fp8, DoubleRow layout [q, u, i, x16], kk = 2u + i):
            # fcl[64*hl + c, (16*(2u+i)) + 6*b + m]
            #   = rt[64*b + c, 48*b + 12*(2u+i) + 6*hl + m]   (h = 2*kk + hl)
            rt = rts[p]
            fcl = small.tile([128, 64], F8, tag="fcl", name=f"fcl{p}")
            fcl_g = fcl.rearrange("q (kk x) -> q kk x", kk=4)
            rt_v = rt.rearrange("q (b kk hl m) -> q b kk hl m", b=2, kk=4, hl=2)
            for hl in range(2):
                for b in range(2):
                    dst = fcl_g[64 * hl : 64 * hl + 64, :, 6 * b : 6 * b + 6]
                    src = rt_v[64 * b : 64 * b + 64, b, :, hl, :]
                    nc.vector.tensor_copy(out=dst, in_=src)
            fcls[p] = fcl

        def emit_fc(p):
            # o2[q, d] = sum_kk fcl[:, 16kk : 16kk+12].T @ Wo[128kk : +128, :]
            # as 2 DoubleRow matmuls (kk pairs (0,1) and (2,3))
            fcl = fcls[p]
            o2 = sm_psum.tile([16, D], F32, tag="sm", name=f"o2_{p}")
            for u in range(2):
                lhsT = fcl[:, 32 * u : 32 * u + 32].rearrange(
                    "q (i m) -> q i m", i=2
                )
                rhs = wod_sb[:, 384 * u : 384 * u + 384].rearrange(
                    "q (i m) -> q i m", i=2
                )
                nc.tensor.matmul(
                    out=o2, lhsT=lhsT, rhs=rhs, perf_mode=DR,
                    start=(u == 0), stop=(u == 1),
                )
            o2s[p] = o2

        def emit_out_add(p):
            nc.vector.tensor_add(
                out=out_all[:, D * p : D * (p + 1)], in0=o2s[p][0:12, :],
                in1=zbo_sb[:, D * p : D * (p + 1)],
            )
            out_rp = bass.AP(
                tensor=out_h.ap().tensor, offset=p * 2 * M * D,
                ap=[[D, 2 * M], [1, D]],
            )
            nc.sync.dma_start(out=out_rp, in_=out_all[:, D * p : D * (p + 1)])

        out_all = const.tile([12, NPAIR * D], F32)

        emit_dma(0)
        emit_dma(1)
        emit_dma(2)
        emit_qt2(0)
        # Flattened software pipeline: pair p's QK waves interleave with
        # pair p-1's trailing AV / softmax-tail work, keeping the PE stream
        # homogeneous and giving exp() three QK-waves of slack before its
        # output is consumed by AV.
        for p in range(NPAIR):
            if p + 3 < NPAIR:
                emit_dma(p + 3)
            if p + 1 < NPAIR:
                emit_qt2(p + 1)
            rsums[p] = rs_psum.tile([96, 129], F32, tag="rs", name=f"rsum{p}")

            for w in range(NW):
                at = at_psum.tile([128, 5 * 96], F32, tag="at", name=f"at{p}_{w}")
                emit_qk_wave(p, w, at)
                emit_exp_wave(p, w, at)
                if w == 0 and p > 0:
                    for j2 in (5, 6):
                        emit_av_dr(p - 1, j2)
                if w == 1 and p > 0:
                    for j2 in (7, 8):
                        emit_av_dr(p - 1, j2)
                if w == 2 and p > 0:
                    for j2 in (9, 10, 11):
                        emit_av_dr(p - 1, j2)
                    emit_av_last(p - 1)
                    emit_norm(p - 1)
                if w == 3:
                    if p > 0:
                        emit_rt(p - 1)
                        emit_fcl(p - 1)
                    for j2 in (0, 1):
                        emit_av_dr(p, j2)
                if w == 4:
                    if p > 0:
                        emit_fc(p - 1)
                        emit_out_add(p - 1)
                    for j2 in (2, 3, 4):
                        emit_av_dr(p, j2)

        p = NPAIR - 1
        for j2 in (5, 6, 7, 8, 9, 10, 11):
            emit_av_dr(p, j2)
        emit_av_last(p)
        emit_norm(p)
        emit_rt(p)
        emit_fcl(p)
        emit_fc(p)
        emit_out_add(p)

    return nc


def get_nc() -> bass.Bass:
    if "nc" not in _CACHE:
        nc = _build_nc()
        # The PJRT exec path serializes nc.m as-is; run Bacc's legalization
        # (wait splitting, register allocation, ...) explicitly.
        nc.finalize()
        _CACHE["nc"] = nc
    return _CACHE["nc"]


def make_in_maps(x, z, Wq, bq, Wo, bo):
    """Host-side prep + sharding into per-core input maps."""
    x = np.asarray(x, dtype=np.float32)
    z = np.asarray(z, dtype=np.float32)
    Wq = np.asarray(Wq, dtype=np.float32)
    bq = np.asarray(bq, dtype=np.float32)
    Wo = np.asarray(Wo, dtype=np.float32)
    bo = np.asarray(bo, dtype=np.float32)

    x_f8 = x.astype(FP8)
    wq_s = (Wq * np.float32(QSC)).astype(FP8)  # QSC folded out in exp scale
    bqt = (bq * np.float32(QSC)).reshape(4, 128).T  # [128, 4], chunk ii col ii
    zbo = (z + bo[None, None, :]).astype(np.float32)
    ident = np.eye(96, dtype=BF16)
    # wod[p, 384u + 192i + d] = Wo[128*(2u+i) + p, d]
    wod = np.ascontiguousarray(
        Wo.astype(FP8).reshape(4, 128, D).transpose(1, 0, 2).reshape(128, 4 * D)
    )
    x_flat = x_f8.reshape(B, C, HW)

    in_maps = []
    for i in range(N_CORES):
        s = slice(i * BPC, (i + 1) * BPC)
        # zt[d, 6*b_local + m] = z[core_base + b_local, m, d]
        zt = z[s].reshape(BPC * M, D).T.astype(FP8)
        qpk = np.zeros((128, 1216), dtype=FP8)
        qpk[:, 0:96] = zt[0:128]
        qpk[0:64, 96:192] = zt[128:192]
        qpk[:, 192:704] = wq_s[0:128]
        qpk[0:64, 704:1216] = wq_s[128:192]

        xc = x_flat[s]  # [16, 64, 3136] fp8
        # xx = [xf | xt] per pair:
        # xf[c2, n] = x[2p + c2//64, c2%64, n]
        xf = xc.reshape(NPAIR, 2 * C, HW)
        # xt[r, 144j + c2] = x[2p + c2//64, c2%64, 128j + r]; col 128 = 1.0
        xp = xf.transpose(0, 2, 1)  # [8, 3136, 128]
        xpad = np.zeros((NPAIR, NCHUNK * 128, 128), dtype=FP8)
        xpad[:, 0:HW, :] = xp
        xj = xpad.reshape(NPAIR, NCHUNK, 128, 128).transpose(0, 2, 1, 3)
        xt = np.zeros((NPAIR, 128, NCHUNK, CSTRIDE), dtype=FP8)
        xt[:, :, :, 0:128] = xj
        xt[:, :, :, 128] = FP8(1.0)
        xx = np.concatenate([xf, xt.reshape(NPAIR, 128, XTW)], axis=2)

        in_maps.append(
            {
                "xx": np.ascontiguousarray(xx),
                "qpk": qpk,
                "bqt": np.ascontiguousarray(bqt),
                "zbo": zbo[s],
                "ident": ident,
                "wod": wod,
            }
        )
    return in_maps


def kernel(**inputs) -> np.ndarray:
    nc = get_nc()
    in_maps = make_in_maps(
        inputs["x"], inputs["z"], inputs["Wq"], inputs["bq"],
        inputs["Wo"], inputs["bo"],
    )
    res = run_bass_kernel_spmd(nc, in_maps, list(range(N_CORES)))
    out = np.concatenate(
        [np.asarray(res.results[i]["out"]) for i in range(N_CORES)], axis=0
    )
    return out.astype(np.float32)
